# revision 1
# baseline (speedup 1.0000x reference)
"""Trainium2 Bass kernel for naive causal MHA (dense transformer block).

Problem: x[2, 2048, 1024], per-head QKV (16 heads, head_dim 64), causal
softmax attention, concat heads, output projection.

Sharding (8 NeuronCores, tensor-parallel over heads):
  - core c computes QKV + attention for heads {2c, 2c+1} over both batches,
    entirely in a transposed layout: scores are built as [keys, queries] so
    the softmax denominator comes from an extra ones-column in V and the
    attention output lands directly in the [head_dim, seq] layout the output
    projection needs as its stationary operand. No on-device transposes.
  - an 8-way AllToAll reshards y from head-split to row-split,
  - each core computes a disjoint 512-row slice of y @ Wout + bout.
The host only slices/transposes inputs and concatenates the 8 row-slices.

All matmuls run in float32r (single-pass FP22 on the PE array).
"""

import contextlib
import ctypes
import sys
import types

import numpy as np

import concourse.bacc as bacc
import concourse.mybir as mybir
import concourse.tile as tile
from concourse.bass import ds

N_CORES = 8
B = 2
S = 2048
D = 1024
HD = 64
N_HEADS = 16

DT = mybir.dt.float32
DTR = mybir.dt.float32r

SC = 512          # seq chunk (moving-operand width)
N_SC = S // SC    # 4
N_DC = D // 128   # 8 contraction chunks
N_SB = S // 128   # 16 seq 128-blocks


def _f32r(ap):
    return ap.bitcast(DTR)


def _mask_np():
    """mask4[j] for the expT tile [t=128, q=512] whose t-block is the j-th
    diagonal block of the q-chunk: q-subblocks < j are zero, == j are
    upper-triangular (keep t <= q), > j are ones."""
    m = np.zeros((4, 128, SC), dtype=np.float32)
    tri = np.triu(np.ones((128, 128), dtype=np.float32))
    for j in range(4):
        m[j, :, j * 128 : (j + 1) * 128] = tri
        m[j, :, (j + 1) * 128 :] = 1.0
    return m


def _build_program(dbg=False):
    nc = bacc.Bacc(
        "TRN2", target_bir_lowering=False, debug=False, num_devices=N_CORES
    )

    xt_d = nc.dram_tensor("xt", [B, D, S], DT, kind="ExternalInput").ap()
    wq_d = nc.dram_tensor("wq", [D, 128], DT, kind="ExternalInput").ap()
    wk_d = nc.dram_tensor("wk", [D, 128], DT, kind="ExternalInput").ap()
    wv_d = nc.dram_tensor("wv", [D, 128], DT, kind="ExternalInput").ap()
    bq_d = nc.dram_tensor("bq", [128, 1], DT, kind="ExternalInput").ap()
    bk_d = nc.dram_tensor("bk", [128, 1], DT, kind="ExternalInput").ap()
    bv_d = nc.dram_tensor("bv", [1, 128], DT, kind="ExternalInput").ap()
    wout_d = nc.dram_tensor("wout", [D, D], DT, kind="ExternalInput").ap()
    bout_d = nc.dram_tensor("bout", [1, D], DT, kind="ExternalInput").ap()
    out_d = nc.dram_tensor("out", [512, D], DT, kind="ExternalOutput").ap()

    y_part = nc.dram_tensor("y_part", [8, 128, 512], DT)
    y_all = nc.dram_tensor("y_all", [8, 128, 512], DT)
    if dbg:
        dbg_qT = nc.dram_tensor("dbg_qT", [B, 128, S], DT, kind="ExternalOutput").ap()
        dbg_kT = nc.dram_tensor("dbg_kT", [B, 128, S], DT, kind="ExternalOutput").ap()
        dbg_v = nc.dram_tensor("dbg_v", [B, 128, N_SB * 2 * 65], DT, kind="ExternalOutput").ap()
        dbg_yp = nc.dram_tensor("dbg_yp", [8, 128, 512], DT, kind="ExternalOutput").ap()
        dbg_ya = nc.dram_tensor("dbg_ya", [8, 128, 512], DT, kind="ExternalOutput").ap()

    mask_d = nc.inline_tensor(_mask_np(), name="mask4")
    ones_d = nc.inline_tensor(
        np.ones((128, N_SB, 2, 1), dtype=np.float32), name="vones"
    )

    with tile.TileContext(nc) as tc, contextlib.ExitStack() as ctx:
        const = ctx.enter_context(tc.tile_pool(name="const", bufs=1))
        xt_pool = ctx.enter_context(tc.tile_pool(name="xt", bufs=9))
        qk_pool = ctx.enter_context(tc.tile_pool(name="qk", bufs=2))
        v_pool = ctx.enter_context(tc.tile_pool(name="vp", bufs=2))
        exp_pool = ctx.enter_context(tc.tile_pool(name="expp", bufs=6))
        zr_pool = ctx.enter_context(tc.tile_pool(name="zr", bufs=2))
        zb_pool = ctx.enter_context(tc.tile_pool(name="zb", bufs=3))
        yts_pool = ctx.enter_context(tc.tile_pool(name="yts", bufs=3))
        yg_pool = ctx.enter_context(tc.tile_pool(name="yg", bufs=1))
        outs_pool = ctx.enter_context(tc.tile_pool(name="outs", bufs=3))
        psum = ctx.enter_context(tc.tile_pool(name="psum", bufs=2, space="PSUM"))
        dram_pool = ctx.enter_context(tc.tile_pool(name="dram", bufs=4, space="DRAM"))

        # ---- constants into SBUF ----
        wq_sb = const.tile([128, N_DC, 128], DT)
        nc.sync.dma_start(out=_f32r(wq_sb), in_=_f32r(wq_d.rearrange("(c p) e -> p c e", p=128)))
        wk_sb = const.tile([128, N_DC, 128], DT)
        nc.sync.dma_start(out=_f32r(wk_sb), in_=_f32r(wk_d.rearrange("(c p) e -> p c e", p=128)))
        wv_sb = const.tile([128, N_DC, 128], DT)
        nc.sync.dma_start(out=_f32r(wv_sb), in_=_f32r(wv_d.rearrange("(c p) e -> p c e", p=128)))
        wout_sb = const.tile([128, N_DC, D], DT)
        nc.sync.dma_start(out=_f32r(wout_sb), in_=_f32r(wout_d.rearrange("(c p) e -> p c e", p=128)))
        bq_sb = const.tile([128, 1], DT)
        nc.sync.dma_start(out=bq_sb, in_=bq_d)
        bk_sb = const.tile([128, 1], DT)
        nc.sync.dma_start(out=bk_sb, in_=bk_d)
        bv_bc = const.tile([128, 128], DT)
        nc.sync.dma_start(out=bv_bc, in_=bv_d.to_broadcast([128, 128]))
        bout_bc = const.tile([128, D], DT)
        nc.sync.dma_start(out=bout_bc, in_=bout_d.to_broadcast([128, D]))
        mask_sb = const.tile([128, 4, SC], DT)
        nc.sync.dma_start(out=mask_sb, in_=mask_d.ap().transpose([1, 0, 2]))

        for b in range(B):
            # ---- QKV projection for batch b ----
            qT = qk_pool.tile([128, S], DT, tag="qT")
            kT = qk_pool.tile([128, S], DT, tag="kT")
            v_sb = v_pool.tile([128, N_SB, 2, 65], DT)
            nc.sync.dma_start(
                out=_f32r(v_sb[:, :, :, 64:65]), in_=_f32r(ones_d.ap())
            )
            for sc in range(N_SC):
                xts = []
                for dc in range(N_DC):
                    xt = xt_pool.tile([128, SC], DT)
                    nc.sync.dma_start(
                        out=_f32r(xt),
                        in_=_f32r(xt_d[b, ds(dc * 128, 128), ds(sc * SC, SC)]),
                    )
                    xts.append(xt)
                psq = psum.tile([128, SC], DT, tag="psq", bufs=1)
                for dc in range(N_DC):
                    nc.tensor.matmul(
                        psq, _f32r(wq_sb[:, dc, :]), _f32r(xts[dc]),
                        start=(dc == 0), stop=(dc == N_DC - 1),
                    )
                nc.vector.tensor_scalar_add(
                    out=_f32r(qT[:, ds(sc * SC, SC)]), in0=psq, scalar1=bq_sb
                )
                psk = psum.tile([128, SC], DT, tag="psk", bufs=1)
                for dc in range(N_DC):
                    nc.tensor.matmul(
                        psk, _f32r(wk_sb[:, dc, :]), _f32r(xts[dc]),
                        start=(dc == 0), stop=(dc == N_DC - 1),
                    )
                nc.vector.tensor_scalar_add(
                    out=_f32r(kT[:, ds(sc * SC, SC)]), in0=psk, scalar1=bk_sb
                )
                for j4 in range(4):
                    psv = psum.tile([128, 128], DT, tag="psv", bufs=1)
                    for dc in range(N_DC):
                        nc.tensor.matmul(
                            psv,
                            _f32r(xts[dc][:, ds(j4 * 128, 128)]),
                            _f32r(wv_sb[:, dc, :]),
                            start=(dc == 0), stop=(dc == N_DC - 1),
                        )
                    sb_i = sc * 4 + j4
                    nc.vector.tensor_add(
                        out=_f32r(v_sb[:, sb_i, :, 0:64]),
                        in0=psv.rearrange("p (h e) -> p h e", h=2),
                        in1=bv_bc.rearrange("p (h e) -> p h e", h=2),
                    )

            # ---- attention for batch b (2 heads) ----
            # copy head-1 rows down to base partition 0: all matmul operands
            # at base 0 (base-64 operand pairs misbehave on HW)
            qT1 = qk_pool.tile([64, S], DT, tag="qT1", bufs=1)
            nc.sync.dma_start(out=_f32r(qT1), in_=_f32r(qT[64:128, :]))
            kT1 = qk_pool.tile([64, S], DT, tag="kT1", bufs=1)
            nc.sync.dma_start(out=_f32r(kT1), in_=_f32r(kT[64:128, :]))
            for qc in range(N_SC):
                ntb = 4 * qc + 4
                # interleave both heads' score->exp->AV chains so the PE's
                # in-order AV matmuls hide the other head's exp latency
                psy0 = psum.tile([65, SC], DT, tag="psy", bufs=2)
                psy1 = psum.tile([65, SC], DT, tag="psy", bufs=2)
                psys = [psy0, psy1]
                for tb in range(ntb):
                    exs = []
                    for h in range(2):
                        qTh, kTh = (qT, kT) if h == 0 else (qT1, kT1)
                        pss = psum.tile([128, SC], DT, tag="pss", bufs=3)
                        nc.tensor.matmul(
                            pss,
                            _f32r(kTh[0:64, ds(tb * 128, 128)]),
                            _f32r(qTh[0:64, ds(qc * SC, SC)]),
                            start=True, stop=True,
                        )
                        ex = exp_pool.tile([128, SC], DT)
                        nc.scalar.activation(
                            out=_f32r(ex), in_=pss,
                            func=mybir.ActivationFunctionType.Exp,
                            scale=0.125,
                        )
                        j = tb - 4 * qc
                        if j >= 0:
                            nc.vector.tensor_mul(
                                out=_f32r(ex), in0=_f32r(ex), in1=mask_sb[:, j, :]
                            )
                        exs.append(ex)
                    for h in range(2):
                        nc.tensor.matmul(
                            psys[h], _f32r(v_sb[:, tb, h, :]), _f32r(exs[h]),
                            start=(tb == 0), stop=(tb == ntb - 1),
                        )
                for h in range(2):
                    hb = 64 * h
                    psy = psys[h]
                    # normalize: row 64 of psy is sum(exp)
                    zr = zr_pool.tile([65, SC], DT)
                    nc.vector.reciprocal(out=zr[64:65, :], in_=psy[64:65, :])
                    zd = dram_pool.tile([1, SC], DT)
                    nc.sync.dma_start(out=zd, in_=zr[64:65, :])
                    zb = zb_pool.tile([64, SC], DT)
                    nc.sync.dma_start(out=zb, in_=zd.to_broadcast([64, SC]))
                    yts = yts_pool.tile([64, SC], DT)
                    nc.vector.tensor_mul(out=yts, in0=psy[0:64, :], in1=zb)
                    nc.sync.dma_start(
                        out=y_part.ap()[b * 4 + qc, ds(hb, 64), :], in_=yts
                    )

            if dbg:
                nc.sync.dma_start(out=dbg_qT[b], in_=qT)
                nc.sync.dma_start(out=dbg_kT[b], in_=kT)
                nc.sync.dma_start(
                    out=dbg_v[b], in_=v_sb.rearrange("p a b c -> p (a b c)")
                )

        # ---- reshard: head-split -> row-split ----
        nc.gpsimd.collective_compute(
            "AllToAll",
            mybir.AluOpType.bypass,
            replica_groups=[list(range(N_CORES))],
            ins=[y_part.ap()],
            outs=[y_all.ap()],
        )

        if dbg:
            nc.sync.dma_start(out=dbg_yp, in_=y_part.ap())
            nc.sync.dma_start(out=dbg_ya, in_=y_all.ap())

        # ---- output projection for this core's 512 rows ----
        ygs = []
        for ec in range(8):
            yg = yg_pool.tile([128, 512], DT, tag=f"yg{ec}")
            nc.sync.dma_start(out=_f32r(yg), in_=_f32r(y_all.ap()[ec]))
            ygs.append(yg)
        for sb in range(4):
            for ch in range(2):
                pso = psum.tile([128, SC], DT, tag="pss", bufs=3)
                for ec in range(8):
                    nc.tensor.matmul(
                        pso,
                        _f32r(ygs[ec][:, ds(sb * 128, 128)]),
                        _f32r(wout_sb[:, ec, ds(ch * SC, SC)]),
                        start=(ec == 0), stop=(ec == 7),
                    )
                ot = outs_pool.tile([128, SC], DT)
                nc.vector.tensor_add(
                    out=ot, in0=pso, in1=bout_bc[:, ds(ch * SC, SC)]
                )
                nc.sync.dma_start(
                    out=out_d[ds(sb * 128, 128), ds(ch * SC, SC)], in_=ot
                )

    nc.compile()
    return nc


_NC_CACHE = None


def _get_program():
    global _NC_CACHE
    if _NC_CACHE is None:
        _NC_CACHE = _build_program()
    return _NC_CACHE


def make_in_maps(x, Wqkv, bqkv, Wout, bout):
    x = np.asarray(x, dtype=np.float32)
    Wqkv = np.asarray(Wqkv, dtype=np.float32)
    bqkv = np.asarray(bqkv, dtype=np.float32)
    Wout = np.asarray(Wout, dtype=np.float32)
    bout = np.asarray(bout, dtype=np.float32)

    xt = np.ascontiguousarray(x.transpose(0, 2, 1))  # [B, D, S]
    wout = np.ascontiguousarray(Wout)
    bout2 = np.ascontiguousarray(bout.reshape(1, D))

    in_maps = []
    for c in range(N_CORES):
        h0, h1 = 2 * c, 2 * c + 1
        wq = np.ascontiguousarray(
            np.concatenate([Wqkv[h0, :, 0:64], Wqkv[h1, :, 0:64]], axis=1)
        )
        wk = np.ascontiguousarray(
            np.concatenate([Wqkv[h0, :, 64:128], Wqkv[h1, :, 64:128]], axis=1)
        )
        wv = np.ascontiguousarray(
            np.concatenate([Wqkv[h0, :, 128:192], Wqkv[h1, :, 128:192]], axis=1)
        )
        bq = np.ascontiguousarray(
            np.concatenate([bqkv[h0, 0:64], bqkv[h1, 0:64]]).reshape(128, 1)
        )
        bk = np.ascontiguousarray(
            np.concatenate([bqkv[h0, 64:128], bqkv[h1, 64:128]]).reshape(128, 1)
        )
        bv = np.ascontiguousarray(
            np.concatenate([bqkv[h0, 128:192], bqkv[h1, 128:192]]).reshape(1, 128)
        )
        in_maps.append(
            {
                "xt": xt,
                "wq": wq,
                "wk": wk,
                "wv": wv,
                "bq": bq,
                "bk": bk,
                "bv": bv,
                "wout": wout,
                "bout": bout2,
            }
        )
    return in_maps


def assemble(results):
    full = np.empty((N_CORES * 512, D), dtype=np.float32)
    for c in range(N_CORES):
        full[512 * c : 512 * (c + 1)] = results[c]["out"]
    return full.reshape(B, S, D)


def _install_ntff_hook():
    """The agent image's antenv lacks axon_hooks; provide it so
    run_bass_kernel_spmd(trace=True) can NTFF-profile via libaxon."""
    if "antenv.axon_hooks" in sys.modules:
        return
    so_path = "/opt/axon/libaxon_pjrt.so"
    try:
        lib = ctypes.CDLL(so_path)
        lib.axon_start_nrt_profile.argtypes = [
            ctypes.POINTER(ctypes.c_int64),
            ctypes.c_size_t,
        ]
        lib.axon_start_nrt_profile.restype = ctypes.c_int64
        lib.axon_stop_nrt_profile.argtypes = [ctypes.c_char_p]
        lib.axon_stop_nrt_profile.restype = ctypes.c_int64
    except (OSError, AttributeError):
        return

    @contextlib.contextmanager
    def _hook(output_dir, device_ids):
        import jax

        jax.devices()
        if device_ids:
            ids = (ctypes.c_int64 * len(device_ids))(*device_ids)
            rc = lib.axon_start_nrt_profile(ids, len(device_ids))
        else:
            rc = lib.axon_start_nrt_profile(None, 0)
        if rc != 0:
            raise RuntimeError(f"axon_start_nrt_profile rc={rc}")
        try:
            yield
        finally:
            n = lib.axon_stop_nrt_profile(str(output_dir).encode())
            if n < 0:
                raise RuntimeError(f"axon_stop_nrt_profile rc={n}")

    mod = types.ModuleType("antenv.axon_hooks")
    mod.get_axon_ntff_profile_hook = lambda: _hook
    mod.set_axon_ntff_profile_hook = lambda h: None
    sys.modules["antenv.axon_hooks"] = mod


def run(inputs, trace=False):
    """Run on the 8 NeuronCores. Returns (output, BassKernelResults)."""
    from concourse.bass_utils import run_bass_kernel_spmd

    if trace:
        _install_ntff_hook()
    nc = _get_program()
    in_maps = make_in_maps(**inputs)
    res = run_bass_kernel_spmd(
        nc, in_maps, core_ids=list(range(N_CORES)), trace=trace
    )
    return assemble(res.results), res


def kernel(x, Wqkv, bqkv, Wout, bout):
    out, _ = run(
        {"x": x, "Wqkv": Wqkv, "bqkv": bqkv, "Wout": Wout, "bout": bout},
        trace=False,
    )
    return out



# revision 17
# speedup vs baseline: 1.7939x; 1.7939x over previous
"""Trainium2 Bass kernel for naive causal MHA (dense transformer block).

Problem: x[2, 2048, 1024], per-head QKV (16 heads, head_dim 64), causal
softmax attention, concat heads, output projection.

Sharding (8 NeuronCores, tensor-parallel over heads):
  - core c computes QKV + attention for heads {2c, 2c+1} over both batches
    in a transposed layout: scores are built as [keys, queries]; the softmax
    denominator comes from an extra ones-column in V.
  - per-batch 8-way AllToAll reshards y from head-split to token-split
    (256 tokens per core per batch); batch-0's AllToAll overlaps batch-1
    compute.
  - each core computes out-proj for its 2x256 token rows.

All matmul operands are bf16 (tolerance 2e-2 gives plenty of headroom);
PSUM accumulation is fp32. Exps are batched 2 score-tiles per ACTIVATE
(PSUM group tiles spanning 2 banks). Softmax normalization is taken off
the PE critical path: psy is evacuated to SBUF immediately, then
reciprocal_approx_fast + DRAM-broadcast + multiply run behind the PE.
"""

import contextlib
import ctypes
import sys
import types

import numpy as np
import ml_dtypes

import concourse.bacc as bacc
import concourse.mybir as mybir
import concourse.tile as tile
from concourse.bass import ds

N_CORES = 8
B = 2
S = 2048
D = 1024
HD = 64
N_HEADS = 16

F32 = mybir.dt.float32
BF = mybir.dt.bfloat16
NPBF = ml_dtypes.bfloat16

SC = 512          # seq chunk (moving-operand width)
N_SC = S // SC    # 4
N_DC = D // 128   # 8 contraction chunks
N_SB = S // 128   # 16 seq 128-blocks
TPC = 256         # tokens per core per batch after reshard


def _mask_np():
    """mask[j] for the exp tile [t=128, q=512] whose t-block is the j-th
    diagonal block of the q-chunk: q-subblocks < j are zero, == j are
    upper-triangular (keep t <= q), > j are ones."""
    m = np.zeros((4, 128, SC), dtype=np.float32)
    tri = np.triu(np.ones((128, 128), dtype=np.float32))
    for j in range(4):
        m[j, :, j * 128 : (j + 1) * 128] = tri
        m[j, :, (j + 1) * 128 :] = 1.0
    return m.astype(NPBF)


def _build_program(dbg=False):
    nc = bacc.Bacc(
        "TRN2", target_bir_lowering=False, debug=False, num_devices=N_CORES
    )

    xt_d = nc.dram_tensor("xt", [B, D, S], BF, kind="ExternalInput").ap()
    wq_d = nc.dram_tensor("wq", [D, 128], BF, kind="ExternalInput").ap()
    wk_d = nc.dram_tensor("wk", [D, 128], BF, kind="ExternalInput").ap()
    wv_d = nc.dram_tensor("wv", [D, 128], BF, kind="ExternalInput").ap()
    bq_d = nc.dram_tensor("bq", [128, 1], F32, kind="ExternalInput").ap()
    bk_d = nc.dram_tensor("bk", [128, 1], F32, kind="ExternalInput").ap()
    bv_d = nc.dram_tensor("bv", [1, 128], F32, kind="ExternalInput").ap()
    wout_d = nc.dram_tensor("wout", [D, D], BF, kind="ExternalInput").ap()
    bout_d = nc.dram_tensor("bout", [1, D], F32, kind="ExternalInput").ap()
    out_d = nc.dram_tensor("out", [2 * TPC, D], F32, kind="ExternalOutput").ap()

    y_part = [nc.dram_tensor(f"y_part{b}", [8, 128, TPC], BF) for b in range(B)]
    y_all = [nc.dram_tensor(f"y_all{b}", [8, 128, TPC], BF) for b in range(B)]
    if dbg:
        dbg_qT = nc.dram_tensor("dbg_qT", [B, 128, S], BF, kind="ExternalOutput").ap()
        dbg_kT = nc.dram_tensor("dbg_kT", [B, 128, S], BF, kind="ExternalOutput").ap()
        dbg_v = nc.dram_tensor("dbg_v", [B, 128, N_SB * 2 * 68], BF, kind="ExternalOutput").ap()
        dbg_yz = nc.dram_tensor("dbg_yz", [B, N_SC, 2, 65, SC], F32, kind="ExternalOutput").ap()
        dbg_ya = nc.dram_tensor("dbg_ya", [B, 8, 128, TPC], BF, kind="ExternalOutput").ap()

    mask_d = nc.inline_tensor(_mask_np(), name="mask4")
    ones_d = nc.inline_tensor(
        np.ones((128, N_SB, 2, 1), dtype=NPBF), name="vones"
    )

    with tile.TileContext(nc) as tc, contextlib.ExitStack() as ctx:
        const = ctx.enter_context(tc.tile_pool(name="const", bufs=1))
        xt_pool = ctx.enter_context(tc.tile_pool(name="xt", bufs=10))
        qk_pool = ctx.enter_context(tc.tile_pool(name="qk", bufs=2))
        v_pool = ctx.enter_context(tc.tile_pool(name="vp", bufs=2))
        exp_pool = ctx.enter_context(tc.tile_pool(name="expp", bufs=6))
        yz_pool = ctx.enter_context(tc.tile_pool(name="yz", bufs=4))
        zr_pool = ctx.enter_context(tc.tile_pool(name="zr", bufs=4))
        zb_pool = ctx.enter_context(tc.tile_pool(name="zb", bufs=4))
        yts_pool = ctx.enter_context(tc.tile_pool(name="yts", bufs=4))
        yg_pool = ctx.enter_context(tc.tile_pool(name="yg", bufs=1))
        outs_pool = ctx.enter_context(tc.tile_pool(name="outs", bufs=3))
        psum = ctx.enter_context(tc.tile_pool(name="psum", bufs=2, space="PSUM"))
        dram_pool = ctx.enter_context(tc.tile_pool(name="dram", bufs=4, space="DRAM"))

        # ---- constants into SBUF ----
        wq_sb = const.tile([128, N_DC, 128], BF)
        nc.sync.dma_start(out=wq_sb, in_=wq_d.rearrange("(c p) e -> p c e", p=128))
        wk_sb = const.tile([128, N_DC, 128], BF)
        nc.sync.dma_start(out=wk_sb, in_=wk_d.rearrange("(c p) e -> p c e", p=128))
        wv_sb = const.tile([128, N_DC, 128], BF)
        nc.sync.dma_start(out=wv_sb, in_=wv_d.rearrange("(c p) e -> p c e", p=128))
        wout_sb = const.tile([128, N_DC, D], BF)
        nc.sync.dma_start(out=wout_sb, in_=wout_d.rearrange("(c p) e -> p c e", p=128))
        bq_sb = const.tile([128, 1], F32)
        nc.sync.dma_start(out=bq_sb, in_=bq_d)
        bk_sb = const.tile([128, 1], F32)
        nc.sync.dma_start(out=bk_sb, in_=bk_d)
        bv_bc = const.tile([128, 128], F32)
        nc.sync.dma_start(out=bv_bc, in_=bv_d.to_broadcast([128, 128]))
        bout_bc = const.tile([128, D], F32)
        nc.sync.dma_start(out=bout_bc, in_=bout_d.to_broadcast([128, D]))
        mask_sb = const.tile([128, 4, SC], BF)
        nc.sync.dma_start(out=mask_sb, in_=mask_d.ap().transpose([1, 0, 2]))

        for b in range(B):
            # ---- QKV projection for batch b ----
            qT = qk_pool.tile([128, S], BF, tag="qT")
            kT = qk_pool.tile([128, S], BF, tag="kT")
            qT1 = qk_pool.tile([64, S], BF, tag="qT1")
            kT1 = qk_pool.tile([64, S], BF, tag="kT1")
            # inner dim padded 65->68 so every (sb, h) block is 8B-aligned
            v_sb = v_pool.tile([128, N_SB, 2, 68], BF)
            nc.vector.memset(v_sb[:, :, :, 64:66], 1.0)
            for sc in range(N_SC):
                xts = []
                for dc in range(N_DC):
                    xt = xt_pool.tile([128, SC], BF)
                    nc.sync.dma_start(
                        out=xt,
                        in_=xt_d[b, ds(dc * 128, 128), ds(sc * SC, SC)],
                    )
                    xts.append(xt)
                psqk = psum.tile([128, 2, SC], F32, tag="pss", bufs=3)
                for dc in range(N_DC):
                    nc.tensor.matmul(
                        psqk[:, 0, :], wq_sb[:, dc, :], xts[dc],
                        start=(dc == 0), stop=(dc == N_DC - 1),
                    )
                for dc in range(N_DC):
                    nc.tensor.matmul(
                        psqk[:, 1, :], wk_sb[:, dc, :], xts[dc],
                        start=(dc == 0), stop=(dc == N_DC - 1),
                    )
                nc.vector.tensor_scalar_add(
                    out=qT[:, ds(sc * SC, SC)], in0=psqk[:, 0, :], scalar1=bq_sb
                )
                nc.vector.tensor_scalar_add(
                    out=kT[:, ds(sc * SC, SC)], in0=psqk[:, 1, :], scalar1=bk_sb
                )
                # head-1 rows to a base-0 tile (base-64 matmul operand pairs
                # misbehave on HW)
                nc.sync.dma_start(
                    out=qT1[:, ds(sc * SC, SC)], in_=qT[64:128, ds(sc * SC, SC)]
                )
                nc.sync.dma_start(
                    out=kT1[:, ds(sc * SC, SC)], in_=kT[64:128, ds(sc * SC, SC)]
                )
                psv = psum.tile([128, SC], F32, tag="psy", bufs=2)
                for j4 in range(4):
                    for dc in range(N_DC):
                        nc.tensor.matmul(
                            psv[:, ds(j4 * 128, 128)],
                            xts[dc][:, ds(j4 * 128, 128)],
                            wv_sb[:, dc, :],
                            start=(dc == 0), stop=(dc == N_DC - 1),
                        )
                for j4 in range(4):
                    sb_i = sc * 4 + j4
                    nc.vector.tensor_add(
                        out=v_sb[:, sb_i, :, 0:64],
                        in0=psv[:, ds(j4 * 128, 128)].rearrange(
                            "p (h e) -> p h e", h=2
                        ),
                        in1=bv_bc.rearrange("p (h e) -> p h e", h=2),
                    )

            # ---- attention for batch b (2 heads) ----
            for qc in range(N_SC):
                ntb = 4 * qc + 4
                ng = ntb // 2
                psy_t = [
                    psum.tile([128, SC], F32, tag="psy", bufs=2, name=f"psy{h}")
                    for h in range(2)
                ]
                pend = None  # (g, [ex_h0, ex_h1]) awaiting AV matmuls

                def emit_avs(g, exs):
                    for h in range(2):
                        for j in range(2):
                            tb = 2 * g + j
                            nc.tensor.matmul(
                                psy_t[h][0:65, :],
                                v_sb[:, tb, h, 0:65],
                                exs[h][:, j, :],
                                start=(tb == 0), stop=(tb == ntb - 1),
                            )

                for g in range(ng):
                    exs = []
                    for h in range(2):
                        qTh, kTh = (qT, kT) if h == 0 else (qT1, kT1)
                        pss = psum.tile([128, 2, SC], F32, tag="pss", bufs=3)
                        for j in range(2):
                            tb = 2 * g + j
                            nc.tensor.matmul(
                                pss[:, j, :],
                                kTh[0:64, ds(tb * 128, 128)],
                                qTh[0:64, ds(qc * SC, SC)],
                                start=True, stop=True,
                            )
                        ex = exp_pool.tile([128, 2, SC], BF)
                        nc.scalar.activation(
                            out=ex, in_=pss,
                            func=mybir.ActivationFunctionType.Exp,
                            scale=0.125,
                        )
                        if 2 * g >= 4 * qc:  # diagonal group
                            mj = 2 * (g - 2 * qc)
                            nc.vector.tensor_mul(
                                out=ex, in0=ex, in1=mask_sb[:, mj : mj + 2, :]
                            )
                        exs.append(ex)
                    if pend is not None:
                        emit_avs(*pend)
                    pend = (g, exs)
                emit_avs(*pend)

                # ---- normalize off the PE critical path ----
                for h in range(2):
                    yz = yz_pool.tile([65, SC], F32)
                    nc.vector.tensor_copy(out=yz, in_=psy_t[h][0:65, :])
                    if dbg:
                        nc.sync.dma_start(out=dbg_yz[b, qc, h], in_=yz)
                    # full-tile (base-partition-0) op: the custom-DVE
                    # reciprocal misbehaves on a single partition at base 64
                    zr = zr_pool.tile([65, SC], F32)
                    nc.vector.reciprocal_approx_fast(out=zr, in_=yz)
                    zd = dram_pool.tile([1, SC], F32)
                    nc.sync.dma_start(out=zd, in_=zr[64:65, :])
                    zb = zb_pool.tile([64, SC], F32)
                    nc.sync.dma_start(out=zb, in_=zd.to_broadcast([64, SC]))
                    yts = yts_pool.tile([64, SC], BF)
                    # on GpSimd: keeps the zb DMA wait out of the in-order
                    # DVE queue (mask muls there are on the AV critical path)
                    nc.gpsimd.tensor_mul(out=yts, in0=yz[0:64, :], in1=zb)
                    for t2 in range(2):
                        nc.sync.dma_start(
                            out=y_part[b].ap()[
                                2 * qc + t2, ds(h * 64, 64), :
                            ],
                            in_=yts[:, ds(t2 * TPC, TPC)],
                        )

            if dbg:
                nc.sync.dma_start(out=dbg_qT[b], in_=qT)
                nc.sync.dma_start(out=dbg_kT[b], in_=kT)
                nc.sync.dma_start(
                    out=dbg_v[b], in_=v_sb.rearrange("p a b c -> p (a b c)")
                )

            # ---- reshard batch b: head-split -> token-split ----
            nc.gpsimd.collective_compute(
                "AllToAll",
                mybir.AluOpType.bypass,
                replica_groups=[list(range(N_CORES))],
                ins=[y_part[b].ap()],
                outs=[y_all[b].ap()],
            )

        # ---- output projection for this core's 2x256 token rows ----
        for b in range(B):
            ygs = []
            for ec in range(8):
                yg = yg_pool.tile([128, TPC], BF, tag=f"yg{b}_{ec}")
                nc.sync.dma_start(out=yg, in_=y_all[b].ap()[ec])
                if dbg:
                    nc.sync.dma_start(out=dbg_ya[b, ec], in_=yg)
                ygs.append(yg)
            for tb2 in range(2):
                for ch in range(2):
                    pso = psum.tile([128, SC], F32, tag="psy", bufs=2)
                    for ec in range(8):
                        nc.tensor.matmul(
                            pso,
                            ygs[ec][:, ds(tb2 * 128, 128)],
                            wout_sb[:, ec, ds(ch * SC, SC)],
                            start=(ec == 0), stop=(ec == 7),
                        )
                    ot = outs_pool.tile([128, SC], F32)
                    nc.vector.tensor_add(
                        out=ot, in0=pso, in1=bout_bc[:, ds(ch * SC, SC)]
                    )
                    nc.sync.dma_start(
                        out=out_d[ds(b * 2 * 128 + tb2 * 128, 128), ds(ch * SC, SC)],
                        in_=ot,
                    )

    nc.compile()
    return nc


_NC_CACHE = {}


def _get_program(dbg=False):
    if dbg not in _NC_CACHE:
        _NC_CACHE[dbg] = _build_program(dbg)
    return _NC_CACHE[dbg]


def make_in_maps(x, Wqkv, bqkv, Wout, bout):
    x = np.asarray(x, dtype=np.float32)
    Wqkv = np.asarray(Wqkv, dtype=np.float32)
    bqkv = np.asarray(bqkv, dtype=np.float32)
    Wout = np.asarray(Wout, dtype=np.float32)
    bout = np.asarray(bout, dtype=np.float32)

    xt = np.ascontiguousarray(x.transpose(0, 2, 1).astype(NPBF))  # [B, D, S]
    wout = np.ascontiguousarray(Wout.astype(NPBF))
    bout2 = np.ascontiguousarray(bout.reshape(1, D))

    in_maps = []
    for c in range(N_CORES):
        h0, h1 = 2 * c, 2 * c + 1
        wq = np.ascontiguousarray(
            np.concatenate([Wqkv[h0, :, 0:64], Wqkv[h1, :, 0:64]], axis=1).astype(NPBF)
        )
        wk = np.ascontiguousarray(
            np.concatenate([Wqkv[h0, :, 64:128], Wqkv[h1, :, 64:128]], axis=1).astype(NPBF)
        )
        wv = np.ascontiguousarray(
            np.concatenate([Wqkv[h0, :, 128:192], Wqkv[h1, :, 128:192]], axis=1).astype(NPBF)
        )
        bq = np.ascontiguousarray(
            np.concatenate([bqkv[h0, 0:64], bqkv[h1, 0:64]]).reshape(128, 1)
        )
        bk = np.ascontiguousarray(
            np.concatenate([bqkv[h0, 64:128], bqkv[h1, 64:128]]).reshape(128, 1)
        )
        bv = np.ascontiguousarray(
            np.concatenate([bqkv[h0, 128:192], bqkv[h1, 128:192]]).reshape(1, 128)
        )
        in_maps.append(
            {
                "xt": xt,
                "wq": wq,
                "wk": wk,
                "wv": wv,
                "bq": bq,
                "bk": bk,
                "bv": bv,
                "wout": wout,
                "bout": bout2,
            }
        )
    return in_maps


def assemble(results):
    full = np.empty((B, S, D), dtype=np.float32)
    for c in range(N_CORES):
        o = results[c]["out"]
        full[0, TPC * c : TPC * (c + 1)] = o[0:TPC]
        full[1, TPC * c : TPC * (c + 1)] = o[TPC : 2 * TPC]
    return full


def _install_ntff_hook():
    """The agent image's antenv lacks axon_hooks; provide it so
    run_bass_kernel_spmd(trace=True) can NTFF-profile via libaxon."""
    if "antenv.axon_hooks" in sys.modules:
        return
    so_path = "/opt/axon/libaxon_pjrt.so"
    try:
        lib = ctypes.CDLL(so_path)
        lib.axon_start_nrt_profile.argtypes = [
            ctypes.POINTER(ctypes.c_int64),
            ctypes.c_size_t,
        ]
        lib.axon_start_nrt_profile.restype = ctypes.c_int64
        lib.axon_stop_nrt_profile.argtypes = [ctypes.c_char_p]
        lib.axon_stop_nrt_profile.restype = ctypes.c_int64
    except (OSError, AttributeError):
        return

    @contextlib.contextmanager
    def _hook(output_dir, device_ids):
        import jax

        jax.devices()
        if device_ids:
            ids = (ctypes.c_int64 * len(device_ids))(*device_ids)
            rc = lib.axon_start_nrt_profile(ids, len(device_ids))
        else:
            rc = lib.axon_start_nrt_profile(None, 0)
        if rc != 0:
            raise RuntimeError(f"axon_start_nrt_profile rc={rc}")
        try:
            yield
        finally:
            n = lib.axon_stop_nrt_profile(str(output_dir).encode())
            if n < 0:
                raise RuntimeError(f"axon_stop_nrt_profile rc={n}")

    mod = types.ModuleType("antenv.axon_hooks")
    mod.get_axon_ntff_profile_hook = lambda: _hook
    mod.set_axon_ntff_profile_hook = lambda h: None
    sys.modules["antenv.axon_hooks"] = mod


def run(inputs, trace=False, dbg=False):
    """Run on the 8 NeuronCores. Returns (output, BassKernelResults)."""
    from concourse.bass_utils import run_bass_kernel_spmd

    if trace:
        _install_ntff_hook()
    nc = _get_program(dbg)
    in_maps = make_in_maps(**inputs)
    res = run_bass_kernel_spmd(
        nc, in_maps, core_ids=list(range(N_CORES)), trace=trace
    )
    return assemble(res.results), res


def kernel(x, Wqkv, bqkv, Wout, bout):
    out, _ = run(
        {"x": x, "Wqkv": Wqkv, "bqkv": bqkv, "Wout": Wout, "bout": bout},
        trace=False,
    )
    return out
